# revision 27
# baseline (speedup 1.0000x reference)
"""Causal self-attention (B=4, T=2048, C=1024, H=16, Dh=64) on 8 TRN2 NeuronCores.

Sharding: batch-data-parallel x head-tensor-parallel. Core c handles batch
c//2 and heads [8*(c%2), 8*(c%2)+8).  Host sums the two half-head partial
projections per batch.  All matmuls fp16 with fp32 PSUM accumulation.

Per-core schedule interleaves QKV projection chunks, attention chunks and
output-projection chunks so the PE always has projection work while the
scalar engine works through the softmax exps:
  for mc in 0..3:  qkv(mc) ; attention(qc=mc) ; proj(mt in 4mc..4mc+3)

Attention per (head-pair, k-tile): two row-group matmuls (contraction 64)
into one [128,2,512] PSUM tile (separate banks, back-to-back issue), one
exp on ACT for both heads, AV accumulation with a ones-column appended to
V for the softmax denominator.  Softmax division: Z rows staged to
partitions {0,32,64,96}, batched reciprocal, gpsimd partition-broadcast,
DVE multiply.
"""

import sys
from collections import deque

if "/opt/trn_rl_repo" not in sys.path:
    sys.path.insert(0, "/opt/trn_rl_repo")

import numpy as np

B, T, C, H, Dh = 4, 2048, 1024, 16, 64
NCORES = 8
HPC = 8                    # heads per core
KT_C = C // 128            # 8 contraction tiles for the projections
TKT = T // 128             # 16 key tiles per batch
QW = 512                   # query chunk width
QC = T // QW               # 4 query chunks
SCALE = 1.0 / np.sqrt(Dh)
VPAD = 66                  # V head-block stride (65 used)
DEBUG_YT = False

_cache = {}


def _build(has_v_bias: bool):
    import concourse.tile as tile
    from concourse import bacc, mybir

    f32 = mybir.dt.float32
    f16 = mybir.dt.float16
    EXP = mybir.ActivationFunctionType.Exp

    nc = bacc.Bacc("TRN2", target_bir_lowering=False, debug=False,
                   num_devices=NCORES)

    xT16_d = nc.dram_tensor("xT16", [C, T], f16, kind="ExternalInput")
    wqk_d = nc.dram_tensor("w_qk", [C, 1024], f16, kind="ExternalInput")
    wv_d = nc.dram_tensor("w_v", [C, 512], f16, kind="ExternalInput")
    wp_d = nc.dram_tensor("w_p", [512, C], f16, kind="ExternalInput")
    bqk_d = nc.dram_tensor("b_qk", [128, 4, 2], f32, kind="ExternalInput")
    bv_d = nc.dram_tensor("b_v_row", [128, 8, 64], f32, kind="ExternalInput")
    mask_d = nc.dram_tensor("masks16", [128, 4, 2, QW], f16,
                            kind="ExternalInput")
    out_d = nc.dram_tensor("out_p", [T, C], f16, kind="ExternalOutput")
    if DEBUG_YT:
        dbg_d = nc.dram_tensor("dbg_yt", [128, 4 * T], f16,
                               kind="ExternalOutput")

    xT16_t = xT16_d.ap().rearrange("(kt p) m -> p kt m", p=128)
    wqk_t = wqk_d.ap().rearrange("(kt p) n -> p kt n", p=128)
    wv_t = wv_d.ap().rearrange("(kt p) n -> p kt n", p=128)
    wp_t = wp_d.ap().rearrange("(s p) n -> p s n", p=128)

    with tile.TileContext(nc) as tc:
        with tc.tile_pool(name="consts", bufs=1) as consts, \
             tc.tile_pool(name="work", bufs=2) as work, \
             tc.tile_pool(name="ps", bufs=2, space="PSUM") as ps:

            # ---- constants / inputs ----
            # DMA order: first-needed-first so the PE starts within ~5us.
            # HBM is ~358GB/s; the ~8.5MB of inputs stream in behind the
            # first QKV chunk's compute.
            wqk_sb = consts.tile([128, KT_C, 1024], f16)
            wv_sb = consts.tile([128, KT_C, 512], f16)
            wp_sb = consts.tile([128, 4, 1024], f16)
            bqk_sb = consts.tile([128, 4, 2], f32)
            mask_sb = consts.tile([128, 4, 2, QW], f16)
            xT16_sb = consts.tile([128, KT_C, T], f16)

            for kt in range(KT_C):                      # x chunk 0 (1MB)
                nc.sync.dma_start(xT16_sb[:, kt, 0:512], xT16_t[:, kt, 0:512])
            nc.sync.dma_start(bqk_sb[:], bqk_d.ap())
            for hp in range(4):                         # wqk in hp blocks
                nc.sync.dma_start(wqk_sb[:, :, hp * 256:(hp + 1) * 256],
                                  wqk_t[:, :, hp * 256:(hp + 1) * 256])
            nc.sync.dma_start(wv_sb[:], wv_t)
            nc.sync.dma_start(mask_sb[:], mask_d.ap())
            if has_v_bias:
                bv_sb = consts.tile([128, 8, 64], f32)
                nc.sync.dma_start(bv_sb[:], bv_d.ap())
            for mc in range(1, 4):                      # remaining x chunks
                for kt in range(KT_C):
                    nc.sync.dma_start(
                        xT16_sb[:, kt, mc * 512:(mc + 1) * 512],
                        xT16_t[:, kt, mc * 512:(mc + 1) * 512])
            nc.sync.dma_start(wp_sb[:], wp_t)

            ones64_sb = consts.tile([1, 64], f16)       # rank-1 bcast lhsT
            nc.vector.memset(ones64_sb[:], 1.0)

            QK_sb = consts.tile([128, 4, 2, T], f16)    # [d2, hp, q/k, m]
            Vt = consts.tile([128, TKT, 8, VPAD], f16)  # [k, kt, head, d+1]
            YT = consts.tile([128, 4, T], f16)
            for kt in range(TKT):
                # ones column for the softmax denominator; keep the AP <=3D
                # (higher-rank strided engine APs misbehave on HW)
                nc.vector.memset(Vt[:, kt, :, Dh:Dh + 1], 1.0)

            def qkv_chunk(mc):
                # generator: ONE tag="st" alloc and ~8 matmuls per yield so
                # the interleaver can inject fills in parity-preserving pairs
                for hp in range(4):
                    col = hp * 256
                    for qk in range(2):
                        pq = ps.tile([128, 512], f32, tag="st")
                        for kt in range(KT_C):
                            nc.tensor.matmul(
                                pq[:],
                                wqk_sb[:, kt, col + qk * 128:col + qk * 128 + 128],
                                xT16_sb[:, kt, mc * 512:(mc + 1) * 512],
                                start=(kt == 0), stop=(kt == KT_C - 1))
                        nc.vector.tensor_scalar_add(
                            QK_sb[:, hp, qk, mc * 512:(mc + 1) * 512],
                            pq[:], bqk_sb[:, hp, qk:qk + 1])
                        yield 1810
                for mt in range(4 * mc, 4 * mc + 4):
                    vp = ps.tile([128, 8, 64], f32, tag="st")
                    msl = slice(mt * 128, (mt + 1) * 128)
                    for kt in range(KT_C):
                        nc.tensor.matmul(
                            vp[:], xT16_sb[:, kt, msl],
                            wv_sb[:, kt, :],
                            start=(kt == 0), stop=(kt == KT_C - 1))
                    dst = Vt[:, mt, :, 0:Dh]
                    if has_v_bias:
                        nc.vector.tensor_add(dst, vp[:], bv_sb[:])
                    else:
                        nc.vector.tensor_copy(dst, vp[:])
                    yield 1810

            def division(pair, qc, yts, pe_bcast=False):
                # softmax normalize for one head-pair-group; emitted lazily
                # (interleaved into later work) so its DVE ops never block
                # the next pair's mask ops in the DVE FIFO.  Per-head Z row
                # pulled straight from PSUM on the DVE (keeps ACT free for
                # the exp chain the PE is waiting on).  pe_bcast=True swaps
                # the 1.1us gpsimd partition-broadcast for a 0.2us rank-1
                # PE matmul -- used for the final division on the critical
                # tail, where the PE is otherwise idle (and going cold).
                qsl = slice(qc * QW, (qc + 1) * QW)
                for lh in range(4):
                    hd = 4 * pair + lh
                    hp, hi = hd // 2, hd % 2
                    yt = yts[hp - 2 * pair][hi]
                    # partition_broadcast only honors base-partition-0
                    # sources on HW, so land Z at partition 0 directly
                    zr = work.tile([1, QW], f32, tag="zr", bufs=2)
                    nc.vector.tensor_copy(zr[:], yt[64:65, :])
                    if pe_bcast:
                        rec = work.tile([1, QW], f32, tag="rc", bufs=2)
                        nc.vector.reciprocal_approx_fast(rec[:], zr[:])
                        rec16 = work.tile([1, QW], f16, tag="rh", bufs=2)
                        nc.vector.tensor_copy(rec16[:], rec[:])
                        yield
                        bcp = ps.tile([64, QW], f32, tag="st")
                        nc.tensor.matmul(bcp[:], ones64_sb[0:1, :],
                                         rec16[0:1, :],
                                         start=True, stop=True)
                        # DVE can't take two PSUM operands; stage to SBUF
                        bcv = work.tile([64, QW], f32, tag="bc", bufs=3)
                        nc.vector.tensor_copy(bcv[:], bcp[:])
                        yield
                        nc.vector.tensor_mul(
                            YT[hi * 64:(hi + 1) * 64, hp, qsl],
                            yt[0:64, :], bcv[:])
                        yield
                        continue
                    rec = work.tile([1, QW], f32, tag="rc", bufs=2)
                    nc.vector.reciprocal_approx_fast(rec[:], zr[:])
                    yield
                    bcv = work.tile([64, QW], f32, tag="bc", bufs=3)
                    nc.gpsimd.partition_broadcast(bcv[:], rec[0:1, :])
                    yield
                    nc.vector.tensor_mul(
                        YT[hi * 64:(hi + 1) * 64, hp, qsl],
                        yt[0:64, :], bcv[:])
                    yield

            def att_chunk(qc, state):
                # One-deep software pipeline: the AV pair for k-tile ki is
                # emitted one step late, AFTER the yield point, so fill work
                # injected at the yield lands between QK(ki+1) and AV(ki) in
                # the PE stream -- inside the exp-latency shadow.  Pending
                # divisions live in a deque and are stepped one op per yield
                # (never bulk-drained); the in-flight AV closure crosses
                # pair/chunk boundaries via `state`.
                pendings, pending_av = state
                nkt = 4 * (qc + 1)
                for pair in range(2):
                    yts = []
                    for hp in (2 * pair, 2 * pair + 1):
                        yt0 = ps.tile([65, QW], f32, tag="yt0")
                        yt1 = ps.tile([65, QW], f32, tag="yt1")
                        yts.append((yt0, yt1))
                        # diagonal (masked) k-tiles first: their DVE mask
                        # op then pipelines against the unmasked tail
                        kts = list(range(4 * qc, nkt)) + list(range(4 * qc))
                        for ki, kt in enumerate(kts):
                            ksl = slice(kt * 128, (kt + 1) * 128)
                            r = kt - 4 * qc
                            # columns < 128r of a diagonal tile are fully
                            # causal-masked: skip them in ST/exp/mask/AV.
                            # r=0 (full width) runs first so its start=True
                            # covers the whole accumulator bank.
                            cs = 128 * r if r > 0 else 0
                            w = QW - cs
                            qs2 = slice(qc * QW + cs, (qc + 1) * QW)
                            stp = ps.tile([128, 2, 512], f32, tag="st")
                            nc.tensor.matmul(
                                stp[:, 0, cs:], QK_sb[0:64, hp, 1, ksl],
                                QK_sb[0:64, hp, 0, qs2],
                                start=True, stop=True)
                            nc.tensor.matmul(
                                stp[:, 1, cs:], QK_sb[64:128, hp, 1, ksl],
                                QK_sb[64:128, hp, 0, qs2],
                                start=True, stop=True)
                            pp = work.tile([128, 2, QW], f16, tag="pp",
                                           bufs=8)
                            nc.scalar.activation(
                                pp[:, :, cs:], stp[:, :, cs:],
                                EXP, scale=SCALE)
                            if r >= 0:
                                nc.vector.tensor_mul(
                                    pp[:, :, cs:], pp[:, :, cs:],
                                    mask_sb[:, r, :, cs:])
                            yield (3 * w) // 2 + 134, (2 * w + 352) * 5 // 6
                            if pending_av is not None:
                                pending_av()
                            first, last = (ki == 0), (ki == nkt - 1)

                            def make_av(pp=pp, cs=cs, kt=kt, hp=hp,
                                       yt0=yt0, yt1=yt1, first=first,
                                       last=last):
                                def emit():
                                    for hi in range(2):
                                        nc.tensor.matmul(
                                            (yt0, yt1)[hi][:, cs:],
                                            Vt[:, kt, 2 * hp + hi, 0:Dh + 1],
                                            pp[:, hi, cs:],
                                            start=first, stop=last)
                                return emit
                            pending_av = make_av()
                            if pendings:
                                if next(pendings[0], StopIteration) \
                                        is StopIteration:
                                    pendings.popleft()
                    pendings.append(division(pair, qc, yts))
                return pendings, pending_av

            def proj_chunks(mts):
                # generator over output tiles; proj(mt) needs all of YT's
                # mt*128 columns, i.e. att chunk mt//4 fully divided
                for mt in mts:
                    ot = work.tile([128, 1024], f16, tag="ot", bufs=4)
                    msl = slice(mt * 128, (mt + 1) * 128)
                    for nh in range(2):
                        nsl = slice(nh * 512, (nh + 1) * 512)
                        pj = ps.tile([128, 512], f32, tag="st")
                        for s in range(4):
                            nc.tensor.matmul(
                                pj[:], YT[:, s, msl],
                                wp_sb[:, s, nsl],
                                start=(s == 0), stop=(s == 3))
                        nc.vector.tensor_copy(ot[:, nsl], pj[:])
                        yield 950
                    nc.sync.dma_start(out_d.ap()[msl, :], ot[:])

            def drain(gen):
                for _ in gen:
                    pass

            def interleave(att_gen, fill_gen):
                # credit scheduler: keep emitted PE work slightly ahead of
                # emitted ACT (exp) work so the PE never starves waiting on
                # an exp result.  Fills are injected in PAIRS: an even
                # number of tag="st" allocations preserves the stp buffer
                # rotation parity, so the next QK's stp never lands on a
                # buffer whose exp is still in flight.
                cum_pe, cum_act = 0, 0
                fills = iter(fill_gen) if fill_gen is not None else None
                while True:
                    try:
                        pe, act = next(att_gen)
                    except StopIteration as e:
                        state = e.value
                        break
                    cum_pe += pe
                    cum_act += act
                    while fills is not None and cum_pe < cum_act + 4000:
                        f = next(fills, None)
                        if f is None:
                            fills = None
                            break
                        cum_pe += f
                        # zero-access dummy keeps the stp slot rotation
                        # aligned so the next QK's stp never lands on a
                        # buffer whose exp is still in flight
                        dmy = ps.tile([128, 1], f32, tag="st")
                        del dmy
                if fills is not None:
                    drain(fills)
                return state

            # qkv(0) first; att(mc) overlapped with qkv(mc+1); the last
            # (largest) attention chunk overlapped with proj tiles 0-11
            # (they only need att chunks 0-2); each pair's softmax division
            # is emitted lazily into the following pairs' streams
            drain(qkv_chunk(0))
            state = (deque(), None)
            state = interleave(att_chunk(0, state), qkv_chunk(1))
            state = interleave(att_chunk(1, state), qkv_chunk(2))
            state = interleave(att_chunk(2, state), qkv_chunk(3))
            state = interleave(att_chunk(3, state), proj_chunks(range(9)))
            pendings, pending_av = state
            if pending_av is not None:
                pending_av()
            # Tail: proj tiles 9-11 were held back as PE filler -- they only
            # need qc<=2 divisions, so they interleave 1:1 with the final
            # divisions' DVE/gpsimd steps, keeping the PE warm through the
            # drain without adding any DVE work.
            tgen = proj_chunks(range(9, 12))
            for g in pendings:
                for _ in g:
                    next(tgen, None)
            drain(tgen)
            drain(proj_chunks(range(12, TKT)))

            if DEBUG_YT:
                nc.sync.dma_start(
                    dbg_d.ap().rearrange("p (s m) -> p s m", s=4), YT[:])

    nc.compile()
    return nc


def _get_nc(has_v_bias: bool):
    key = ("nc", has_v_bias)
    if key not in _cache:
        _cache[key] = _build(has_v_bias)
    return _cache[key]


def _make_masks(dtype) -> np.ndarray:
    # masks[p, r, hd, q] = 1.0 where key (128*r + p) <= query q in a QW chunk
    p = np.arange(128)[:, None, None]
    r = np.arange(4)[None, :, None]
    q = np.arange(QW)[None, None, :]
    m = ((128 * r + p) <= q).astype(dtype)            # [128, 4, QW]
    return np.ascontiguousarray(np.repeat(m[:, :, None, :], 2, axis=2))


def kernel(x, W_qkv, b_qkv, W_proj, b_proj):
    from concourse.bass_utils import run_bass_kernel_spmd

    x = np.asarray(x, dtype=np.float32)
    W_qkv = np.asarray(W_qkv, dtype=np.float32)
    b_qkv = np.asarray(b_qkv, dtype=np.float32)
    W_proj = np.asarray(W_proj, dtype=np.float32)
    b_proj = np.asarray(b_proj, dtype=np.float32)

    has_v_bias = bool(np.any(b_qkv[2 * C:] != 0.0))
    nc = _get_nc(has_v_bias)

    masks16 = _make_masks(np.float16)
    xTs = [np.ascontiguousarray(x[b].T).astype(np.float16) for b in range(B)]

    in_maps = []
    for c in range(NCORES):
        b = c // 2
        hbase = (c % 2) * 8
        # wqk cols: (hp, q/k, 2-head block of 128)
        cols = []
        for hp in range(4):
            for qk in range(2):
                base = qk * C + (hbase + 2 * hp) * Dh
                cols.append(W_qkv[:, base:base + 128])
        wqk = np.ascontiguousarray(
            np.concatenate(cols, axis=1).astype(np.float16))
        wv = W_qkv[:, 2 * C + hbase * Dh: 2 * C + hbase * Dh + 512]
        wp = W_proj[hbase * Dh: hbase * Dh + 512, :]
        bqk = np.zeros((128, 4, 2), dtype=np.float32)
        for hp in range(4):
            for qk in range(2):
                base = qk * C + (hbase + 2 * hp) * Dh
                bqk[:, hp, qk] = b_qkv[base:base + 128]
        bv = b_qkv[2 * C + hbase * Dh: 2 * C + hbase * Dh + 512]
        in_maps.append({
            "xT16": xTs[b],
            "w_qk": wqk,
            "w_v": np.ascontiguousarray(wv.astype(np.float16)),
            "w_p": np.ascontiguousarray(wp.astype(np.float16)),
            "b_qk": bqk,
            "b_v_row": np.ascontiguousarray(
                np.broadcast_to(bv.reshape(8, 64)[None], (128, 8, 64))
            ).astype(np.float32),
            "masks16": masks16,
        })

    res = run_bass_kernel_spmd(nc, in_maps, core_ids=list(range(NCORES)),
                               **_cache.get("run_kwargs", {}))
    _cache["last_results"] = res

    out = np.zeros((B, T, C), dtype=np.float32)
    for b in range(B):
        out[b] = (res.results[2 * b]["out_p"].astype(np.float32)
                  + res.results[2 * b + 1]["out_p"].astype(np.float32))
        out[b] += b_proj
    return out



# revision 29
# speedup vs baseline: 1.0109x; 1.0109x over previous
"""Causal self-attention (B=4, T=2048, C=1024, H=16, Dh=64) on 8 TRN2 NeuronCores.

Sharding: batch-data-parallel x head-tensor-parallel. Core c handles batch
c//2 and heads [8*(c%2), 8*(c%2)+8).  Host sums the two half-head partial
projections per batch.  All matmuls fp16 with fp32 PSUM accumulation.

Per-core schedule interleaves QKV projection chunks, attention chunks and
output-projection chunks so the PE always has projection work while the
scalar engine works through the softmax exps:
  for mc in 0..3:  qkv(mc) ; attention(qc=mc) ; proj(mt in 4mc..4mc+3)

Attention per (head-pair, k-tile): two row-group matmuls (contraction 64)
into one [128,2,512] PSUM tile (separate banks, back-to-back issue), one
exp on ACT for both heads, AV accumulation with a ones-column appended to
V for the softmax denominator.  Softmax division: Z rows staged to
partitions {0,32,64,96}, batched reciprocal, gpsimd partition-broadcast,
DVE multiply.
"""

import sys
from collections import deque

if "/opt/trn_rl_repo" not in sys.path:
    sys.path.insert(0, "/opt/trn_rl_repo")

import numpy as np

B, T, C, H, Dh = 4, 2048, 1024, 16, 64
NCORES = 8
HPC = 8                    # heads per core
KT_C = C // 128            # 8 contraction tiles for the projections
TKT = T // 128             # 16 key tiles per batch
QW = 512                   # query chunk width
QC = T // QW               # 4 query chunks
SCALE = 1.0 / np.sqrt(Dh)
VPAD = 66                  # V head-block stride (65 used)
DEBUG_YT = False

_cache = {}


def _build(has_v_bias: bool):
    import concourse.tile as tile
    from concourse import bacc, mybir

    f32 = mybir.dt.float32
    f16 = mybir.dt.float16
    EXP = mybir.ActivationFunctionType.Exp

    nc = bacc.Bacc("TRN2", target_bir_lowering=False, debug=False,
                   num_devices=NCORES)

    xT16_d = nc.dram_tensor("xT16", [C, T], f16, kind="ExternalInput")
    wqk_d = nc.dram_tensor("w_qk", [C, 1024], f16, kind="ExternalInput")
    wv_d = nc.dram_tensor("w_v", [C, 512], f16, kind="ExternalInput")
    wp_d = nc.dram_tensor("w_p", [512, C], f16, kind="ExternalInput")
    bqk_d = nc.dram_tensor("b_qk", [128, 4, 2], f32, kind="ExternalInput")
    bv_d = nc.dram_tensor("b_v_row", [128, 8, 64], f32, kind="ExternalInput")
    mask_d = nc.dram_tensor("masks16", [128, 4, 2, QW], f16,
                            kind="ExternalInput")
    out_d = nc.dram_tensor("out_p", [T, C], f16, kind="ExternalOutput")
    if DEBUG_YT:
        dbg_d = nc.dram_tensor("dbg_yt", [128, 4 * T], f16,
                               kind="ExternalOutput")

    xT16_t = xT16_d.ap().rearrange("(kt p) m -> p kt m", p=128)
    wqk_t = wqk_d.ap().rearrange("(kt p) n -> p kt n", p=128)
    wv_t = wv_d.ap().rearrange("(kt p) n -> p kt n", p=128)
    wp_t = wp_d.ap().rearrange("(s p) n -> p s n", p=128)

    with tile.TileContext(nc) as tc:
        with tc.tile_pool(name="consts", bufs=1) as consts, \
             tc.tile_pool(name="work", bufs=2) as work, \
             tc.tile_pool(name="ps", bufs=2, space="PSUM") as ps:

            # ---- constants / inputs ----
            # DMA order: first-needed-first so the PE starts within ~5us.
            # HBM is ~358GB/s; the ~8.5MB of inputs stream in behind the
            # first QKV chunk's compute.
            wqk_sb = consts.tile([128, KT_C, 1024], f16)
            wv_sb = consts.tile([128, KT_C, 512], f16)
            wp_sb = consts.tile([128, 4, 1024], f16)
            bqk_sb = consts.tile([128, 4, 2], f32)
            mask_sb = consts.tile([128, 4, 2, QW], f16)
            xT16_sb = consts.tile([128, KT_C, T], f16)

            for kt in range(KT_C):                      # x chunk 0 (1MB)
                nc.sync.dma_start(xT16_sb[:, kt, 0:512], xT16_t[:, kt, 0:512])
            nc.sync.dma_start(bqk_sb[:], bqk_d.ap())
            for hp in range(4):                         # wqk in hp blocks
                nc.sync.dma_start(wqk_sb[:, :, hp * 256:(hp + 1) * 256],
                                  wqk_t[:, :, hp * 256:(hp + 1) * 256])
            nc.sync.dma_start(wv_sb[:], wv_t)
            nc.sync.dma_start(mask_sb[:], mask_d.ap())
            if has_v_bias:
                bv_sb = consts.tile([128, 8, 64], f32)
                nc.sync.dma_start(bv_sb[:], bv_d.ap())
            for mc in range(1, 4):                      # remaining x chunks
                for kt in range(KT_C):
                    nc.sync.dma_start(
                        xT16_sb[:, kt, mc * 512:(mc + 1) * 512],
                        xT16_t[:, kt, mc * 512:(mc + 1) * 512])
            nc.sync.dma_start(wp_sb[:], wp_t)

            ones64_sb = consts.tile([1, 64], f16)       # rank-1 bcast lhsT
            nc.vector.memset(ones64_sb[:], 1.0)

            QK_sb = consts.tile([128, 4, 2, T], f16)    # [d2, hp, q/k, m]
            Vt = consts.tile([128, TKT, 8, VPAD], f16)  # [k, kt, head, d+1]
            YT = consts.tile([128, 4, T], f16)
            for kt in range(TKT):
                # ones column for the softmax denominator; keep the AP <=3D
                # (higher-rank strided engine APs misbehave on HW)
                nc.vector.memset(Vt[:, kt, :, Dh:Dh + 1], 1.0)

            def qkv_chunk(mc):
                # generator: ONE tag="st" alloc and ~8 matmuls per yield so
                # the interleaver can inject fills in parity-preserving pairs
                for hp in range(4):
                    col = hp * 256
                    for qk in range(2):
                        pq = ps.tile([128, 512], f32, tag="st")
                        for kt in range(KT_C):
                            nc.tensor.matmul(
                                pq[:],
                                wqk_sb[:, kt, col + qk * 128:col + qk * 128 + 128],
                                xT16_sb[:, kt, mc * 512:(mc + 1) * 512],
                                start=(kt == 0), stop=(kt == KT_C - 1))
                        nc.vector.tensor_scalar_add(
                            QK_sb[:, hp, qk, mc * 512:(mc + 1) * 512],
                            pq[:], bqk_sb[:, hp, qk:qk + 1])
                        yield 1810
                for mt in range(4 * mc, 4 * mc + 4):
                    vp = ps.tile([128, 8, 64], f32, tag="st")
                    msl = slice(mt * 128, (mt + 1) * 128)
                    for kt in range(KT_C):
                        nc.tensor.matmul(
                            vp[:], xT16_sb[:, kt, msl],
                            wv_sb[:, kt, :],
                            start=(kt == 0), stop=(kt == KT_C - 1))
                    dst = Vt[:, mt, :, 0:Dh]
                    if has_v_bias:
                        nc.vector.tensor_add(dst, vp[:], bv_sb[:])
                    else:
                        nc.vector.tensor_copy(dst, vp[:])
                    yield 1810

            def division(pair, qc, yts, pe_bcast=False):
                # softmax normalize for one head-pair-group; emitted lazily
                # (interleaved into later work) so its DVE ops never block
                # the next pair's mask ops in the DVE FIFO.  Per-head Z row
                # pulled straight from PSUM on the DVE (keeps ACT free for
                # the exp chain the PE is waiting on).  pe_bcast=True swaps
                # the 1.1us gpsimd partition-broadcast for a 0.2us rank-1
                # PE matmul -- used for the final division on the critical
                # tail, where the PE is otherwise idle (and going cold).
                qsl = slice(qc * QW, (qc + 1) * QW)
                for lh in range(4):
                    hd = 4 * pair + lh
                    hp, hi = hd // 2, hd % 2
                    yt = yts[hp - 2 * pair][hi]
                    # partition_broadcast only honors base-partition-0
                    # sources on HW, so land Z at partition 0 directly
                    zr = work.tile([1, QW], f32, tag="zr", bufs=2)
                    nc.vector.tensor_copy(zr[:], yt[64:65, :])
                    if pe_bcast:
                        rec = work.tile([1, QW], f32, tag="rc", bufs=2)
                        nc.vector.reciprocal_approx_fast(rec[:], zr[:])
                        rec16 = work.tile([1, QW], f16, tag="rh", bufs=2)
                        nc.vector.tensor_copy(rec16[:], rec[:])
                        yield
                        bcp = ps.tile([64, QW], f32, tag="st")
                        nc.tensor.matmul(bcp[:], ones64_sb[0:1, :],
                                         rec16[0:1, :],
                                         start=True, stop=True)
                        # DVE can't take two PSUM operands; stage to SBUF
                        bcv = work.tile([64, QW], f32, tag="bc", bufs=3)
                        nc.vector.tensor_copy(bcv[:], bcp[:])
                        yield
                        nc.vector.tensor_mul(
                            YT[hi * 64:(hi + 1) * 64, hp, qsl],
                            yt[0:64, :], bcv[:])
                        yield
                        continue
                    rec = work.tile([1, QW], f32, tag="rc", bufs=2)
                    nc.vector.reciprocal_approx_fast(rec[:], zr[:])
                    yield
                    bcv = work.tile([64, QW], f32, tag="bc", bufs=3)
                    nc.gpsimd.partition_broadcast(bcv[:], rec[0:1, :])
                    yield
                    nc.vector.tensor_mul(
                        YT[hi * 64:(hi + 1) * 64, hp, qsl],
                        yt[0:64, :], bcv[:])
                    yield

            def att_chunk(qc, state):
                # One-deep software pipeline: the AV pair for k-tile ki is
                # emitted one step late, AFTER the yield point, so fill work
                # injected at the yield lands between QK(ki+1) and AV(ki) in
                # the PE stream -- inside the exp-latency shadow.  Pending
                # divisions live in a deque and are stepped one op per yield
                # (never bulk-drained); the in-flight AV closure crosses
                # pair/chunk boundaries via `state`.
                pendings, pending_av = state
                nkt = 4 * (qc + 1)
                for pair in range(2):
                    yts = []
                    for hp in (2 * pair, 2 * pair + 1):
                        yt0 = ps.tile([65, QW], f32, tag="yt0")
                        yt1 = ps.tile([65, QW], f32, tag="yt1")
                        yts.append((yt0, yt1))
                        # diagonal (masked) k-tiles first: their DVE mask
                        # op then pipelines against the unmasked tail
                        kts = list(range(4 * qc, nkt)) + list(range(4 * qc))
                        for ki, kt in enumerate(kts):
                            ksl = slice(kt * 128, (kt + 1) * 128)
                            r = kt - 4 * qc
                            # columns < 128r of a diagonal tile are fully
                            # causal-masked: skip them in ST/exp/mask/AV.
                            # r=0 (full width) runs first so its start=True
                            # covers the whole accumulator bank.
                            cs = 128 * r if r > 0 else 0
                            w = QW - cs
                            qs2 = slice(qc * QW + cs, (qc + 1) * QW)
                            stp = ps.tile([128, 2, 512], f32, tag="st")
                            nc.tensor.matmul(
                                stp[:, 0, cs:], QK_sb[0:64, hp, 1, ksl],
                                QK_sb[0:64, hp, 0, qs2],
                                start=True, stop=True)
                            nc.tensor.matmul(
                                stp[:, 1, cs:], QK_sb[64:128, hp, 1, ksl],
                                QK_sb[64:128, hp, 0, qs2],
                                start=True, stop=True)
                            pp = work.tile([128, 2, QW], f16, tag="pp",
                                           bufs=8)
                            nc.scalar.activation(
                                pp[:, :, cs:], stp[:, :, cs:],
                                EXP, scale=SCALE)
                            if r >= 0:
                                # only queries [128r, 128r+128) are partial;
                                # beyond that every key in this tile is
                                # causal-valid -- a 128-wide mask multiply
                                # keeps the exp->mask->AV chain short
                                me = min(128 * r + 128, QW)
                                nc.vector.tensor_mul(
                                    pp[:, :, cs:me], pp[:, :, cs:me],
                                    mask_sb[:, r, :, cs:me])
                            yield (3 * w) // 2 + 134, (2 * w + 352) * 5 // 6
                            if pending_av is not None:
                                pending_av()
                            first, last = (ki == 0), (ki == nkt - 1)

                            def make_av(pp=pp, cs=cs, kt=kt, hp=hp,
                                       yt0=yt0, yt1=yt1, first=first,
                                       last=last):
                                def emit():
                                    for hi in range(2):
                                        nc.tensor.matmul(
                                            (yt0, yt1)[hi][:, cs:],
                                            Vt[:, kt, 2 * hp + hi, 0:Dh + 1],
                                            pp[:, hi, cs:],
                                            start=first, stop=last)
                                return emit
                            pending_av = make_av()
                            if pendings:
                                if next(pendings[0], StopIteration) \
                                        is StopIteration:
                                    pendings.popleft()
                    pendings.append(division(pair, qc, yts))
                return pendings, pending_av

            def proj_chunks(mts):
                # generator over output tiles; proj(mt) needs all of YT's
                # mt*128 columns, i.e. att chunk mt//4 fully divided
                for mt in mts:
                    ot = work.tile([128, 1024], f16, tag="ot", bufs=4)
                    msl = slice(mt * 128, (mt + 1) * 128)
                    for nh in range(2):
                        nsl = slice(nh * 512, (nh + 1) * 512)
                        pj = ps.tile([128, 512], f32, tag="st")
                        for s in range(4):
                            nc.tensor.matmul(
                                pj[:], YT[:, s, msl],
                                wp_sb[:, s, nsl],
                                start=(s == 0), stop=(s == 3))
                        nc.vector.tensor_copy(ot[:, nsl], pj[:])
                        yield 950
                    nc.sync.dma_start(out_d.ap()[msl, :], ot[:])

            def drain(gen):
                for _ in gen:
                    pass

            def interleave(att_gen, fill_gen):
                # credit scheduler: keep emitted PE work slightly ahead of
                # emitted ACT (exp) work so the PE never starves waiting on
                # an exp result.  Fills are injected in PAIRS: an even
                # number of tag="st" allocations preserves the stp buffer
                # rotation parity, so the next QK's stp never lands on a
                # buffer whose exp is still in flight.
                cum_pe, cum_act = 0, 0
                fills = iter(fill_gen) if fill_gen is not None else None
                while True:
                    try:
                        pe, act = next(att_gen)
                    except StopIteration as e:
                        state = e.value
                        break
                    cum_pe += pe
                    cum_act += act
                    while fills is not None and cum_pe < cum_act + 1200:
                        f = next(fills, None)
                        if f is None:
                            fills = None
                            break
                        cum_pe += f
                        # zero-access dummy keeps the stp slot rotation
                        # aligned so the next QK's stp never lands on a
                        # buffer whose exp is still in flight
                        dmy = ps.tile([128, 1], f32, tag="st")
                        del dmy
                if fills is not None:
                    drain(fills)
                return state

            # qkv(0) first; att(mc) overlapped with qkv(mc+1); the last
            # (largest) attention chunk overlapped with proj tiles 0-11
            # (they only need att chunks 0-2); each pair's softmax division
            # is emitted lazily into the following pairs' streams
            drain(qkv_chunk(0))
            state = (deque(), None)
            state = interleave(att_chunk(0, state), qkv_chunk(1))
            state = interleave(att_chunk(1, state), qkv_chunk(2))
            state = interleave(att_chunk(2, state), qkv_chunk(3))
            state = interleave(att_chunk(3, state), proj_chunks(range(9)))
            pendings, pending_av = state
            if pending_av is not None:
                pending_av()
            # Tail: proj tiles 9-11 were held back as PE filler -- they only
            # need qc<=2 divisions, so they interleave 1:1 with the final
            # divisions' DVE/gpsimd steps, keeping the PE warm through the
            # drain without adding any DVE work.
            tgen = proj_chunks(range(9, 12))
            for g in pendings:
                for _ in g:
                    next(tgen, None)
            drain(tgen)
            drain(proj_chunks(range(12, TKT)))

            if DEBUG_YT:
                nc.sync.dma_start(
                    dbg_d.ap().rearrange("p (s m) -> p s m", s=4), YT[:])

    nc.compile()
    return nc


def _get_nc(has_v_bias: bool):
    key = ("nc", has_v_bias)
    if key not in _cache:
        _cache[key] = _build(has_v_bias)
    return _cache[key]


def _make_masks(dtype) -> np.ndarray:
    # masks[p, r, hd, q] = 1.0 where key (128*r + p) <= query q in a QW chunk
    p = np.arange(128)[:, None, None]
    r = np.arange(4)[None, :, None]
    q = np.arange(QW)[None, None, :]
    m = ((128 * r + p) <= q).astype(dtype)            # [128, 4, QW]
    return np.ascontiguousarray(np.repeat(m[:, :, None, :], 2, axis=2))


def kernel(x, W_qkv, b_qkv, W_proj, b_proj):
    from concourse.bass_utils import run_bass_kernel_spmd

    x = np.asarray(x, dtype=np.float32)
    W_qkv = np.asarray(W_qkv, dtype=np.float32)
    b_qkv = np.asarray(b_qkv, dtype=np.float32)
    W_proj = np.asarray(W_proj, dtype=np.float32)
    b_proj = np.asarray(b_proj, dtype=np.float32)

    has_v_bias = bool(np.any(b_qkv[2 * C:] != 0.0))
    nc = _get_nc(has_v_bias)

    masks16 = _make_masks(np.float16)
    xTs = [np.ascontiguousarray(x[b].T).astype(np.float16) for b in range(B)]

    in_maps = []
    for c in range(NCORES):
        b = c // 2
        hbase = (c % 2) * 8
        # wqk cols: (hp, q/k, 2-head block of 128)
        cols = []
        for hp in range(4):
            for qk in range(2):
                base = qk * C + (hbase + 2 * hp) * Dh
                cols.append(W_qkv[:, base:base + 128])
        wqk = np.ascontiguousarray(
            np.concatenate(cols, axis=1).astype(np.float16))
        wv = W_qkv[:, 2 * C + hbase * Dh: 2 * C + hbase * Dh + 512]
        wp = W_proj[hbase * Dh: hbase * Dh + 512, :]
        bqk = np.zeros((128, 4, 2), dtype=np.float32)
        for hp in range(4):
            for qk in range(2):
                base = qk * C + (hbase + 2 * hp) * Dh
                bqk[:, hp, qk] = b_qkv[base:base + 128]
        bv = b_qkv[2 * C + hbase * Dh: 2 * C + hbase * Dh + 512]
        in_maps.append({
            "xT16": xTs[b],
            "w_qk": wqk,
            "w_v": np.ascontiguousarray(wv.astype(np.float16)),
            "w_p": np.ascontiguousarray(wp.astype(np.float16)),
            "b_qk": bqk,
            "b_v_row": np.ascontiguousarray(
                np.broadcast_to(bv.reshape(8, 64)[None], (128, 8, 64))
            ).astype(np.float32),
            "masks16": masks16,
        })

    res = run_bass_kernel_spmd(nc, in_maps, core_ids=list(range(NCORES)),
                               **_cache.get("run_kwargs", {}))
    _cache["last_results"] = res

    out = np.zeros((B, T, C), dtype=np.float32)
    for b in range(B):
        out[b] = (res.results[2 * b]["out_p"].astype(np.float32)
                  + res.results[2 * b + 1]["out_p"].astype(np.float32))
        out[b] += b_proj
    return out

